# revision 23
# baseline (speedup 1.0000x reference)
"""Bass/Tile kernel for nn_Attention_9234179687166 on 8 TRN2 NeuronCores.

Reference computation per batch b (B=32, L=K=D=1024):
    q      = query @ W_in.T                    # [L, D]
    scores = q @ context.T                     # [L, K]
    w      = masked_softmax(scores, mask)      # multiplicative mask + renorm
    mix    = w @ context                       # [L, D]
    out    = tanh(concat([mix, q]) @ W_out.T)  # [L, D]

Sharding: data-parallel over batch, 4 batches per core, weights replicated.

v2 design (vs the v1 PE-transpose kernel): the PE runs *only* the 4
GEMMs.  Inputs are cast to fp16 on the HOST, so every transpose (query,
context, W_in, W_out, softmax weights) is a single XBAR DMA-transpose
straight from DRAM/SBUF (2-byte dtype), input DMA bytes halve, and all
matmul operands are fp16 (fp16 is full-rate on the PE, and numerically
*better* than the old f32r/bf16 mix: 1.8e-3 vs 2.8e-3 rel err, because
the mix/out path gains mantissa bits while the peaked softmax is
insensitive to fp16 scores rounding).

Layouts (standard chunking r = chunk*128 + partition, which is what the
XBAR transpose produces for a 3D [128, n, m] destination):
    W_inT16[d,e], W_outT16[c,d]               (setup, once)
    ctxT16[e,k] / ctx16[k,d]                  (per batch, double-buffered)
    qT16[d,l]                                 (per half, double-buffered)
    wT16[k,l]   <- transpose(e16)             (per l-tile, after exp)

Software pipelining per half: lead masks -> next-half query/ctx DMA
stages -> step2 + masked softmax per l-tile -> step1 of the NEXT half on
the PE (fills the wT transpose latency) -> step4 -> step5 with the
deferred 1/sum(e) folded into the fused combine (per-partition scalar).

HARD-WON HW LESSONS (cost ~2 hours of debugging):
 1. ALL DMAs must go on the SP HWDGE ring.  DMAs issued on the
    Activation ring (nc.scalar.dma_start) do NOT enforce cross-engine
    completion ordering on real TRN2 here - consumers (PE matmuls, DMA
    reads) start early and read stale SBUF.  CoreSim/TimelineSim do not
    model this and pass; only real-HW runs exposed it.
 2. The Tile framework does not emit WAR semaphores for rotating pool
    buffers (tag bufs=N).  Safety comes only from rotation distance vs
    queue skew.  Keep displacement >= a full pipeline stage: masks
    bufs=4 loaded just-in-time inside the lj loop, qT16/ctxT16/ctx16
    double-buffered, o_sb bufs=4 (the out-store DMA reads o_sb
    asynchronously and can be delayed by DMA queue congestion).
 3. Instruction.add_dependency() after emission is invisible to the
    tile scheduler (deps are captured at add_instruction time).
"""

import sys

sys.path.insert(0, "/opt/trn_rl_repo")

import numpy as np

P = 128
D = 1024
TWO_D = 2048
DT = D // P      # 8 tiles over D
LARGE = 4096.0
N_CORES = 8
B_FULL = 32

_prog_cache = {}
last_results = None  # BassKernelResults of the most recent kernel() call


def build_program(nb, L, K=1024, reps=1):
    import bass_rust
    import concourse.mybir as mybir
    import concourse.tile as tile
    from concourse import bacc

    SYNC_DEP = bass_rust.DependencyInfo(sync=True, no_sync=False)

    def order_after(inst, prev):
        # Explicit WAR edge: a DMA that reuses a rotating buffer must wait
        # for the displaced allocation's reader. The tile framework only
        # window-syncs queues, which real-HW timing skew can defeat.
        if inst is None or prev is None:
            return
        i = inst.ins if hasattr(inst, "ins") else inst
        p = prev.ins if hasattr(prev, "ins") else prev
        i.add_dependency(p.name, SYNC_DEP)

    f32 = mybir.dt.float32
    f16 = mybir.dt.float16
    i32 = mybir.dt.int32
    Alu = mybir.AluOpType
    Act = mybir.ActivationFunctionType
    KT = K // P
    LH = min(512, L)      # l-half width (free dim of step1/4 matmuls)
    NHALF = L // LH
    LJ = LH // P          # 128-row l tiles per half
    KH = K // 512         # 512-wide k chunks for the scores matmul

    nc = bacc.Bacc("TRN2", target_bir_lowering=False, debug=False,
                   num_devices=N_CORES)
    q_d = nc.dram_tensor("query", [nb, L, D], f16, kind="ExternalInput")
    c_d = nc.dram_tensor("context", [nb, K, D], f16, kind="ExternalInput")
    m_d = nc.dram_tensor("mask", [nb, L, K], i32, kind="ExternalInput")
    win_d = nc.dram_tensor("W_in", [D, D], f16, kind="ExternalInput")
    wout_d = nc.dram_tensor("W_out", [D, TWO_D], f16, kind="ExternalInput")
    out_d = nc.dram_tensor("out", [nb, L, D], f32, kind="ExternalOutput")

    copy_flip = [0]

    def psum_copy(dst_ap, src_ap):
        # Alternate psum->sbuf copies between DVE and ACT so neither engine
        # serializes the chain behind the matmuls.
        if copy_flip[0] % 2 == 0:
            nc.vector.tensor_copy(dst_ap, src_ap)
        else:
            nc.scalar.activation(dst_ap, src_ap, Act.Copy)
        copy_flip[0] += 1

    with tile.TileContext(nc) as tc:
        with (
            tc.tile_pool(name="wres", bufs=1) as wres,
            tc.tile_pool(name="ctx", bufs=1) as ctxp,
            tc.tile_pool(name="acts", bufs=1) as actsp,
            tc.tile_pool(name="rot", bufs=1) as natp,
            tc.tile_pool(name="sm", bufs=1) as smp,
            tc.tile_pool(name="ps_big", bufs=2, space="PSUM") as ps_big,
            tc.tile_pool(name="ps_mm", bufs=4, space="PSUM") as ps_mm,
        ):
            W_inT16 = wres.tile([P, DT, D], f16)       # [d, ., e]
            W_outT16 = wres.tile([P, 2 * DT, D], f16)  # [c, ., d_out]

            ctx_tiles = {}
            mask_tiles = {}
            s1_last = {}      # unit -> last step1 matmul (reader of qT16)
            scores_last = {}  # batch -> last scores matmul (reader of ctxT16)
            s4_last = {}      # batch -> last step4 matmul (reader of ctx16)
            mask_stts = []   # stt instruction per mask allocation, in order
            mask_loads = []  # load instruction per mask allocation
            MASK_BUFS = 4

            def load_mask(b, h, lj):
                # Masks ride the otherwise-idle Pool/gpsimd DMA queue so no
                # other DMA can ever queue ahead of them.
                if (b, h, lj) in mask_tiles:
                    return
                mi = smp.tile([P, K], i32, tag="mask", bufs=MASK_BUFS)
                l0 = h * LH
                ld = nc.sync.dma_start(
                    mi[:], m_d[b, l0 + lj * P: l0 + (lj + 1) * P, :])
                k = len(mask_loads)
                if k >= MASK_BUFS:
                    order_after(ld, mask_stts[k - MASK_BUFS])
                mask_loads.append(ld)
                mask_tiles[(b, h, lj)] = mi

            def emit_ctx_stage(b, mid_cb=None):
                # Inputs are fp16 in DRAM (host-side cast): the e-major copy
                # is ONE whole-matrix XBAR transpose and the k-major one is
                # ONE rearranged plain load. Double-buffered: batch b+1's
                # stage is emitted at (b, h1)'s top, before s2/s4(b, h1)
                # reads of batch b's buffers.
                ctxT16 = ctxp.tile([P, DT, K], f16, tag="ctxT16", bufs=2)
                ctx16 = ctxp.tile([P, KT, D], f16, tag="ctx16", bufs=2)
                for ki in range(KT):
                    t = nc.sync.dma_start(
                        ctxT16[:, :, ki * P:(ki + 1) * P],
                        c_d[b, ki * P:(ki + 1) * P, :], transpose=True)
                    order_after(t, scores_last.get(b - 2))
                if mid_cb is not None:
                    mid_cb()
                for ki in range(KT):
                    ld = nc.sync.dma_start(
                        ctx16[:, ki, :], c_d[b, ki * P:(ki + 1) * P, :])
                    order_after(ld, s4_last.get(b - 2))
                ctx_tiles[b] = (ctx16, ctxT16)

            def emit_w_in_setup():
                for ei in range(DT):
                    nc.sync.dma_start(
                        W_inT16[:, :, ei * P:(ei + 1) * P],
                        win_d[ei * P:(ei + 1) * P, :], transpose=True)

            def emit_w_out_setup():
                for di in range(DT):
                    nc.sync.dma_start(
                        W_outT16[:, :, di * P:(di + 1) * P],
                        wout_d[di * P:(di + 1) * P, :], transpose=True)

            def emit_query_stage(b, h):
                # XBAR-transpose the fp16 [LH, D] query half from DRAM.
                unit = b * NHALF + h
                l0 = h * LH
                qT16 = actsp.tile([P, DT, LH], f16, tag="qT", bufs=2)
                for lj in range(LJ):
                    t = nc.sync.dma_start(
                        qT16[:, :, lj * P:(lj + 1) * P],
                        q_d[b, l0 + lj * P: l0 + (lj + 1) * P, :],
                        transpose=True)
                    order_after(t, s1_last.get(unit - 2))
                return qT16

            def emit_step1(qT16, unit):
                # qTr16[e, l] = W_inT16.T @ qT16 (fp16, fp32 PSUM accum)
                qTr16 = actsp.tile([P, DT, LH], f16, tag="qTr", bufs=2)
                for ei in range(DT):
                    psq = ps_mm.tile([P, LH], f32, tag="mm")
                    for di in range(DT):
                        mm = nc.tensor.matmul(
                            psq[:],
                            W_inT16[:, di, ei * P:(ei + 1) * P],
                            qT16[:, di, :],
                            start=(di == 0), stop=(di == DT - 1),
                        )
                    psum_copy(qTr16[:, ei, :], psq[:])
                s1_last[unit] = mm
                return qTr16

            def emit_half(b, h, qTr16, stage_cb):
                unit = b * NHALF + h
                ctx16, ctxT16 = ctx_tiles[b]
                l0 = h * LH

                # Two lead masks on SP ahead of everything else; the rest
                # are emitted inside the lj loop (after the stt whose buffer
                # they displace, so rotation stays safe on any timing).
                for lj in range(min(2, LJ)):
                    load_mask(b, h, lj)
                # Next unit's query/ctx DMA stages (no PE work).
                qT_next = stage_cb()

                # ---- step 2 + masked softmax ----
                wT16 = actsp.tile([P, KT, LH], f16, tag="wT", bufs=1)
                rec_all = actsp.tile([P, LJ], f32, tag="recs", bufs=2)
                for lj in range(LJ):
                    if lj + 2 < LJ:
                        load_mask(b, h, lj + 2)
                    pss = ps_big.tile([P, K], f32, tag="big")
                    for ei in range(DT):
                        for kh in range(KH):
                            mm = nc.tensor.matmul(
                                pss[:, kh * 512:(kh + 1) * 512],
                                qTr16[:, ei, lj * P:(lj + 1) * P],
                                ctxT16[:, ei, kh * 512:(kh + 1) * 512],
                                start=(ei == 0), stop=(ei == DT - 1),
                            )
                    scores_last[b] = mm
                    st = smp.tile([P, 4], f32, tag="stats", bufs=2)
                    # u = (s + LARGE) * m in SBUF frees the PSUM tile right
                    # after this op so the next tile's matmuls aren't gated.
                    u_t = smp.tile([P, K], f32, tag="u", bufs=2)
                    stt_i = nc.vector.scalar_tensor_tensor(
                        u_t[:], pss[:], LARGE, mask_tiles.pop((b, h, lj))[:],
                        op0=Alu.add, op1=Alu.mult)
                    mask_stts.append(stt_i)
                    nc.vector.tensor_reduce(
                        st[:, 0:1], u_t[:], axis=mybir.AxisListType.X,
                        op=Alu.max, negate=True)
                    e16 = smp.tile([P, K], f16, tag="e", bufs=2)
                    nc.scalar.activation(
                        e16[:], u_t[:], Act.Exp,
                        bias=st[:, 0:1], accum_out=st[:, 1:2])
                    nc.vector.reciprocal(rec_all[:, lj:lj + 1], st[:, 1:2])
                    # transpose e16 -> wT16 on the ACT queue: its exp
                    # dependency is already satisfied in queue order.
                    nc.sync.dma_start(
                        wT16[:, :, lj * P:(lj + 1) * P], e16[:],
                        transpose=True)

                # Fill the wT DMA latency with the next half's step1 (PE).
                qTr_next = (emit_step1(qT_next, unit + 1)
                            if qT_next is not None else None)

                # ---- step 4: mixT16[d', l] = ctx16.T @ wT16 ----
                mixT16 = actsp.tile([P, DT, LH], f16, tag="mixT", bufs=1)
                for di in range(DT):
                    psm = ps_mm.tile([P, LH], f32, tag="mm")
                    for ki in range(KT):
                        mm = nc.tensor.matmul(
                            psm[:],
                            ctx16[:, ki, di * P:(di + 1) * P],
                            wT16[:, ki, :],
                            start=(ki == 0), stop=(ki == KT - 1),
                        )
                    psum_copy(mixT16[:, di, :], psm[:])
                s4_last[b] = mm

                # ---- step 5: out = tanh(mixT@Wo1 * rec + qTr@Wo2) ----
                for lj in range(LJ):
                    pso_mix = ps_big.tile([P, D], f32, tag="big")
                    pso_q = [ps_mm.tile([P, 512], f32, tag="mm",
                                        name=f"pso_q{dh}")
                             for dh in range(D // 512)]
                    for ci in range(DT):
                        lhs = mixT16[:, ci, lj * P:(lj + 1) * P]
                        for dh in range(D // 512):
                            nc.tensor.matmul(
                                pso_mix[:, dh * 512:(dh + 1) * 512], lhs,
                                W_outT16[:, ci, dh * 512:(dh + 1) * 512],
                                start=(ci == 0), stop=(ci == DT - 1),
                            )
                    for ci in range(DT):
                        lhs = qTr16[:, ci, lj * P:(lj + 1) * P]
                        for dh in range(D // 512):
                            nc.tensor.matmul(
                                pso_q[dh][:], lhs,
                                W_outT16[:, DT + ci, dh * 512:(dh + 1) * 512],
                                start=(ci == 0), stop=(ci == DT - 1),
                            )
                    o_sb = smp.tile([P, D], f32, tag="osb", bufs=4)
                    for dh in range(D // 512):
                        # Only one PSUM operand allowed per DVE op: stage
                        # the q part in SBUF first.
                        sl = slice(dh * 512, (dh + 1) * 512)
                        nc.vector.tensor_copy(o_sb[:, sl], pso_q[dh][:])
                        nc.vector.scalar_tensor_tensor(
                            o_sb[:, sl], pso_mix[:, sl],
                            rec_all[:, lj:lj + 1], o_sb[:, sl],
                            op0=Alu.mult, op1=Alu.add)
                        nc.scalar.activation(o_sb[:, sl], o_sb[:, sl],
                                             Act.Tanh)
                    nc.sync.dma_start(
                        out_d[b, l0 + lj * P: l0 + (lj + 1) * P, :],
                        o_sb[:])
                return qTr_next

            def emit_all():
                qT = emit_query_stage(0, 0)
                emit_w_in_setup()
                load_mask(0, 0, 0)

                def _rest_masks():
                    load_mask(0, 0, 1)

                emit_ctx_stage(0, mid_cb=_rest_masks)
                qTr = emit_step1(qT, 0)
                first = [True]
                for b in range(nb):
                    if b > 0:
                        ctx_tiles.pop(b - 1)
                    for h in range(NHALF):
                        if h + 1 < NHALF:
                            nb_, nh_ = b, h + 1
                        elif b + 1 < nb:
                            nb_, nh_ = b + 1, 0
                        else:
                            nb_, nh_ = None, None

                        def stage_cb(nb_=nb_, nh_=nh_):
                            if nb_ is not None:
                                qT_next = emit_query_stage(nb_, nh_)
                                if nh_ == 0:
                                    emit_ctx_stage(nb_)
                            else:
                                qT_next = None
                            if first[0]:
                                # W_out isn't read until step 5 (~90us in);
                                # emitting it here keeps its 16 transposes
                                # from delaying the first qT prefetch.
                                emit_w_out_setup()
                                first[0] = False
                            return qT_next

                        qTr = emit_half(b, h, qTr, stage_cb)

            if reps == 1:
                emit_all()
            else:
                with tc.For_i(0, reps, 1):
                    emit_all()

    nc.compile()
    return nc


def _get_program(nb, L, reps=1):
    key = (nb, L, reps)
    if key not in _prog_cache:
        _prog_cache[key] = build_program(nb, L, reps=reps)
    return _prog_cache[key]


def kernel(query, context, mask, W_in, W_out):
    from concourse.bass_utils import run_bass_kernel_spmd

    query = np.ascontiguousarray(query, dtype=np.float16)
    context = np.ascontiguousarray(context, dtype=np.float16)
    W_in = np.ascontiguousarray(W_in, dtype=np.float16)
    W_out = np.ascontiguousarray(W_out, dtype=np.float16)
    B, L, _ = query.shape
    mask3 = np.ascontiguousarray(mask.reshape(B, L, -1), dtype=np.int32)

    nb = B // N_CORES
    nc = _get_program(nb, L)
    in_maps = []
    for c in range(N_CORES):
        b0 = c * nb
        in_maps.append({
            "query": query[b0:b0 + nb],
            "context": context[b0:b0 + nb],
            "mask": mask3[b0:b0 + nb],
            "W_in": W_in,
            "W_out": W_out,
        })
    res = run_bass_kernel_spmd(nc, in_maps, core_ids=list(range(N_CORES)))
    global last_results
    last_results = res
    out = np.concatenate([r["out"] for r in res.results], axis=0)
    return out


# revision 27
# speedup vs baseline: 1.0161x; 1.0161x over previous
"""Bass/Tile kernel for nn_Attention_9234179687166 on 8 TRN2 NeuronCores.

Reference computation per batch b (B=32, L=K=D=1024):
    q      = query @ W_in.T                    # [L, D]
    scores = q @ context.T                     # [L, K]
    w      = masked_softmax(scores, mask)      # multiplicative mask + renorm
    mix    = w @ context                       # [L, D]
    out    = tanh(concat([mix, q]) @ W_out.T)  # [L, D]

Sharding: data-parallel over batch, 4 batches per core, weights replicated.

v2 design (vs the v1 PE-transpose kernel): the PE runs *only* the 4
GEMMs.  Inputs are cast to fp16 on the HOST, so every transpose (query,
context, W_in, W_out, softmax weights) is a single XBAR DMA-transpose
straight from DRAM/SBUF (2-byte dtype), input DMA bytes halve, and all
matmul operands are fp16 (fp16 is full-rate on the PE, and numerically
*better* than the old f32r/bf16 mix: 1.8e-3 vs 2.8e-3 rel err, because
the mix/out path gains mantissa bits while the peaked softmax is
insensitive to fp16 scores rounding).

Layouts (standard chunking r = chunk*128 + partition, which is what the
XBAR transpose produces for a 3D [128, n, m] destination):
    W_inT16[d,e], W_outT16[c,d]               (setup, once)
    ctxT16[e,k] / ctx16[k,d]                  (per batch, double-buffered)
    qT16[d,l]                                 (per half, double-buffered)
    wT16[k,l]   <- transpose(e16)             (per l-tile, after exp)

Software pipelining per half: lead masks -> next-half query/ctx DMA
stages -> step2 + masked softmax per l-tile -> step1 of the NEXT half on
the PE (fills the wT transpose latency) -> step4 -> step5 with the
deferred 1/sum(e) folded into the fused combine (per-partition scalar).

HARD-WON HW LESSONS (cost ~2 hours of debugging):
 1. ALL DMAs must go on the SP HWDGE ring.  DMAs issued on the
    Activation ring (nc.scalar.dma_start) do NOT enforce cross-engine
    completion ordering on real TRN2 here - consumers (PE matmuls, DMA
    reads) start early and read stale SBUF.  CoreSim/TimelineSim do not
    model this and pass; only real-HW runs exposed it.
 2. The Tile framework does not emit WAR semaphores for rotating pool
    buffers (tag bufs=N).  Safety comes only from rotation distance vs
    queue skew.  Keep displacement >= a full pipeline stage: masks
    bufs=4 loaded just-in-time inside the lj loop, qT16/ctxT16/ctx16
    double-buffered, o_sb bufs=4 (the out-store DMA reads o_sb
    asynchronously and can be delayed by DMA queue congestion).
 3. Instruction.add_dependency() after emission is invisible to the
    tile scheduler (deps are captured at add_instruction time).
"""

import sys

sys.path.insert(0, "/opt/trn_rl_repo")

import numpy as np

P = 128
D = 1024
TWO_D = 2048
DT = D // P      # 8 tiles over D
LARGE = 4096.0
N_CORES = 8
B_FULL = 32

_prog_cache = {}
last_results = None  # BassKernelResults of the most recent kernel() call


def build_program(nb, L, K=1024, reps=1):
    import bass_rust
    import concourse.mybir as mybir
    import concourse.tile as tile
    from concourse import bacc

    SYNC_DEP = bass_rust.DependencyInfo(sync=True, no_sync=False)

    def order_after(inst, prev):
        # Explicit WAR edge: a DMA that reuses a rotating buffer must wait
        # for the displaced allocation's reader. The tile framework only
        # window-syncs queues, which real-HW timing skew can defeat.
        if inst is None or prev is None:
            return
        i = inst.ins if hasattr(inst, "ins") else inst
        p = prev.ins if hasattr(prev, "ins") else prev
        i.add_dependency(p.name, SYNC_DEP)

    f32 = mybir.dt.float32
    f16 = mybir.dt.float16
    i32 = mybir.dt.int32
    Alu = mybir.AluOpType
    Act = mybir.ActivationFunctionType
    KT = K // P
    LH = min(512, L)      # l-half width (free dim of step1/4 matmuls)
    NHALF = L // LH
    LJ = LH // P          # 128-row l tiles per half
    KH = K // 512         # 512-wide k chunks for the scores matmul

    nc = bacc.Bacc("TRN2", target_bir_lowering=False, debug=False,
                   num_devices=N_CORES)
    q_d = nc.dram_tensor("query", [nb, L, D], f16, kind="ExternalInput")
    c_d = nc.dram_tensor("context", [nb, K, D], f16, kind="ExternalInput")
    m_d = nc.dram_tensor("mask", [nb, L, K], i32, kind="ExternalInput")
    win_d = nc.dram_tensor("W_in", [D, D], f16, kind="ExternalInput")
    wout_d = nc.dram_tensor("W_out", [D, TWO_D], f16, kind="ExternalInput")
    out_d = nc.dram_tensor("out", [nb, L, D], f32, kind="ExternalOutput")

    copy_flip = [0]

    def psum_copy(dst_ap, src_ap):
        # Alternate psum->sbuf copies between DVE and ACT so neither engine
        # serializes the chain behind the matmuls.
        if copy_flip[0] % 2 == 0:
            nc.vector.tensor_copy(dst_ap, src_ap)
        else:
            nc.scalar.activation(dst_ap, src_ap, Act.Copy)
        copy_flip[0] += 1

    with tile.TileContext(nc) as tc:
        with (
            tc.tile_pool(name="wres", bufs=1) as wres,
            tc.tile_pool(name="ctx", bufs=1) as ctxp,
            tc.tile_pool(name="acts", bufs=1) as actsp,
            tc.tile_pool(name="rot", bufs=1) as natp,
            tc.tile_pool(name="sm", bufs=1) as smp,
            tc.tile_pool(name="ps_big", bufs=2, space="PSUM") as ps_big,
            tc.tile_pool(name="ps_mm", bufs=4, space="PSUM") as ps_mm,
        ):
            W_inT16 = wres.tile([P, DT, D], f16)       # [d, ., e]
            W_outT16 = wres.tile([P, 2 * DT, D], f16)  # [c, ., d_out]

            ctx_tiles = {}
            mask_tiles = {}
            s1_last = {}      # unit -> last step1 matmul (reader of qT16)
            scores_last = {}  # batch -> last scores matmul (reader of ctxT16)
            s4_last = {}      # batch -> last step4 matmul (reader of ctx16)
            mask_stts = []   # stt instruction per mask allocation, in order
            mask_loads = []  # load instruction per mask allocation
            MASK_BUFS = 4

            def load_mask(b, h, lj):
                # Masks ride the otherwise-idle Pool/gpsimd DMA queue so no
                # other DMA can ever queue ahead of them.
                if (b, h, lj) in mask_tiles:
                    return
                mi = smp.tile([P, K], i32, tag="mask", bufs=MASK_BUFS)
                l0 = h * LH
                ld = nc.sync.dma_start(
                    mi[:], m_d[b, l0 + lj * P: l0 + (lj + 1) * P, :])
                k = len(mask_loads)
                if k >= MASK_BUFS:
                    order_after(ld, mask_stts[k - MASK_BUFS])
                mask_loads.append(ld)
                mask_tiles[(b, h, lj)] = mi

            def emit_ctx_stage(b, mid_cb=None):
                # Inputs are fp16 in DRAM (host-side cast): the e-major copy
                # is ONE whole-matrix XBAR transpose and the k-major one is
                # ONE rearranged plain load. Double-buffered: batch b+1's
                # stage is emitted at (b, h1)'s top, before s2/s4(b, h1)
                # reads of batch b's buffers.
                ctxT16 = ctxp.tile([P, DT, K], f16, tag="ctxT16", bufs=2)
                ctx16 = ctxp.tile([P, KT, D], f16, tag="ctx16", bufs=2)
                for ki in range(KT):
                    t = nc.sync.dma_start(
                        ctxT16[:, :, ki * P:(ki + 1) * P],
                        c_d[b, ki * P:(ki + 1) * P, :], transpose=True)
                    order_after(t, scores_last.get(b - 2))
                if mid_cb is not None:
                    mid_cb()
                for ki in range(KT):
                    ld = nc.sync.dma_start(
                        ctx16[:, ki, :], c_d[b, ki * P:(ki + 1) * P, :])
                    order_after(ld, s4_last.get(b - 2))
                ctx_tiles[b] = (ctx16, ctxT16)

            def emit_w_in_setup():
                for ei in range(DT):
                    nc.sync.dma_start(
                        W_inT16[:, :, ei * P:(ei + 1) * P],
                        win_d[ei * P:(ei + 1) * P, :], transpose=True)

            def emit_w_out_setup():
                for di in range(DT):
                    nc.sync.dma_start(
                        W_outT16[:, :, di * P:(di + 1) * P],
                        wout_d[di * P:(di + 1) * P, :], transpose=True)

            def emit_query_stage(b, h):
                # XBAR-transpose the fp16 [LH, D] query half from DRAM.
                unit = b * NHALF + h
                l0 = h * LH
                qT16 = actsp.tile([P, DT, LH], f16, tag="qT", bufs=2)
                for lj in range(LJ):
                    t = nc.sync.dma_start(
                        qT16[:, :, lj * P:(lj + 1) * P],
                        q_d[b, l0 + lj * P: l0 + (lj + 1) * P, :],
                        transpose=True)
                    order_after(t, s1_last.get(unit - 2))
                return qT16

            def emit_step1(qT16, unit):
                # qTr16[e, l] = W_inT16.T @ qT16 (fp16, fp32 PSUM accum)
                qTr16 = actsp.tile([P, DT, LH], f16, tag="qTr", bufs=2)
                for ei in range(DT):
                    psq = ps_mm.tile([P, LH], f32, tag="mm")
                    for di in range(DT):
                        mm = nc.tensor.matmul(
                            psq[:],
                            W_inT16[:, di, ei * P:(ei + 1) * P],
                            qT16[:, di, :],
                            start=(di == 0), stop=(di == DT - 1),
                        )
                    psum_copy(qTr16[:, ei, :], psq[:])
                s1_last[unit] = mm
                return qTr16

            def emit_half(b, h, qTr16, stage_cb, late_cb=None):
                unit = b * NHALF + h
                ctx16, ctxT16 = ctx_tiles[b]
                l0 = h * LH

                # Two lead masks on SP ahead of everything else; the rest
                # are emitted inside the lj loop (after the stt whose buffer
                # they displace, so rotation stays safe on any timing).
                for lj in range(min(2, LJ)):
                    load_mask(b, h, lj)
                # Next unit's query/ctx DMA stages (no PE work).
                qT_next = stage_cb()

                # ---- step 2 + masked softmax ----
                wT16 = actsp.tile([P, KT, LH], f16, tag="wT", bufs=1)
                rec_all = actsp.tile([P, LJ], f32, tag="recs", bufs=2)
                e16s = []
                for lj in range(LJ):
                    if lj + 2 < LJ:
                        load_mask(b, h, lj + 2)
                    pss = ps_big.tile([P, K], f32, tag="big")
                    for ei in range(DT):
                        for kh in range(KH):
                            mm = nc.tensor.matmul(
                                pss[:, kh * 512:(kh + 1) * 512],
                                qTr16[:, ei, lj * P:(lj + 1) * P],
                                ctxT16[:, ei, kh * 512:(kh + 1) * 512],
                                start=(ei == 0), stop=(ei == DT - 1),
                            )
                    scores_last[b] = mm
                    st = smp.tile([P, 4], f32, tag="stats", bufs=2)
                    # u = (s + LARGE) * m in SBUF frees the PSUM tile right
                    # after this op so the next tile's matmuls aren't gated.
                    u_t = smp.tile([P, K], f32, tag="u", bufs=1)
                    stt_i = nc.vector.scalar_tensor_tensor(
                        u_t[:], pss[:], LARGE, mask_tiles.pop((b, h, lj))[:],
                        op0=Alu.add, op1=Alu.mult)
                    mask_stts.append(stt_i)
                    nc.vector.tensor_reduce(
                        st[:, 0:1], u_t[:], axis=mybir.AxisListType.X,
                        op=Alu.max, negate=True)
                    e16 = smp.tile([P, K], f16, tag="e", bufs=4)
                    nc.scalar.activation(
                        e16[:], u_t[:], Act.Exp,
                        bias=st[:, 0:1], accum_out=st[:, 1:2])
                    nc.vector.reciprocal(rec_all[:, lj:lj + 1], st[:, 1:2])
                    e16s.append(e16)

                # wT transposes batched after the softmax loop: lj0-2's
                # exps are long done, so these barely block the SP queue
                # (inside the loop each one stalled SP on its exp, delaying
                # every later prefetch DMA behind it).
                for lj in range(LJ):
                    nc.sync.dma_start(
                        wT16[:, :, lj * P:(lj + 1) * P], e16s[lj][:],
                        transpose=True)
                # Bulk prefetch stages (W_out setup, next batch's ctx) go
                # AFTER the wT transposes on SP: they have >=half-a-unit of
                # slack, while wT gates step 4 directly.
                if late_cb is not None:
                    late_cb()

                # Fill the wT DMA latency with the next half's step1 (PE).
                qTr_next = (emit_step1(qT_next, unit + 1)
                            if qT_next is not None else None)

                # ---- step 4: mixT16[d', l] = ctx16.T @ wT16 ----
                mixT16 = actsp.tile([P, DT, LH], f16, tag="mixT", bufs=1)
                for di in range(DT):
                    psm = ps_mm.tile([P, LH], f32, tag="mm")
                    for ki in range(KT):
                        mm = nc.tensor.matmul(
                            psm[:],
                            ctx16[:, ki, di * P:(di + 1) * P],
                            wT16[:, ki, :],
                            start=(ki == 0), stop=(ki == KT - 1),
                        )
                    psum_copy(mixT16[:, di, :], psm[:])
                s4_last[b] = mm

                # ---- step 5: out = tanh(mixT@Wo1 * rec + qTr@Wo2) ----
                o_sbs = []
                for lj in range(LJ):
                    pso_mix = ps_big.tile([P, D], f32, tag="big")
                    pso_q = [ps_mm.tile([P, 512], f32, tag="mm",
                                        name=f"pso_q{dh}")
                             for dh in range(D // 512)]
                    for ci in range(DT):
                        lhs = mixT16[:, ci, lj * P:(lj + 1) * P]
                        for dh in range(D // 512):
                            nc.tensor.matmul(
                                pso_mix[:, dh * 512:(dh + 1) * 512], lhs,
                                W_outT16[:, ci, dh * 512:(dh + 1) * 512],
                                start=(ci == 0), stop=(ci == DT - 1),
                            )
                    for ci in range(DT):
                        lhs = qTr16[:, ci, lj * P:(lj + 1) * P]
                        for dh in range(D // 512):
                            nc.tensor.matmul(
                                pso_q[dh][:], lhs,
                                W_outT16[:, DT + ci, dh * 512:(dh + 1) * 512],
                                start=(ci == 0), stop=(ci == DT - 1),
                            )
                    o_sb = smp.tile([P, D], f32, tag="osb", bufs=4)
                    for dh in range(D // 512):
                        # Only one PSUM operand allowed per DVE op: stage
                        # the q part in SBUF first.
                        sl = slice(dh * 512, (dh + 1) * 512)
                        nc.vector.tensor_copy(o_sb[:, sl], pso_q[dh][:])
                        nc.vector.scalar_tensor_tensor(
                            o_sb[:, sl], pso_mix[:, sl],
                            rec_all[:, lj:lj + 1], o_sb[:, sl],
                            op0=Alu.mult, op1=Alu.add)
                        nc.scalar.activation(o_sb[:, sl], o_sb[:, sl],
                                             Act.Tanh)
                    o_sbs.append(o_sb)
                # Stores batched at the half's end: all tanhs are done so
                # none of them stalls the SP queue head mid-step5.
                for lj, o_sb in enumerate(o_sbs):
                    nc.sync.dma_start(
                        out_d[b, l0 + lj * P: l0 + (lj + 1) * P, :],
                        o_sb[:])
                return qTr_next

            def emit_all():
                qT = emit_query_stage(0, 0)
                emit_w_in_setup()
                load_mask(0, 0, 0)

                def _rest_masks():
                    load_mask(0, 0, 1)

                emit_ctx_stage(0, mid_cb=_rest_masks)
                qTr = emit_step1(qT, 0)
                first = [True]
                for b in range(nb):
                    if b > 0:
                        ctx_tiles.pop(b - 1)
                    for h in range(NHALF):
                        if h + 1 < NHALF:
                            nb_, nh_ = b, h + 1
                        elif b + 1 < nb:
                            nb_, nh_ = b + 1, 0
                        else:
                            nb_, nh_ = None, None

                        def stage_cb(nb_=nb_, nh_=nh_):
                            if nb_ is None:
                                return None
                            return emit_query_stage(nb_, nh_)

                        def late_cb(nb_=nb_, nh_=nh_):
                            if first[0]:
                                emit_w_out_setup()
                                first[0] = False
                            if nb_ is not None and nh_ == 0:
                                emit_ctx_stage(nb_)

                        qTr = emit_half(b, h, qTr, stage_cb, late_cb)

            if reps == 1:
                emit_all()
            else:
                with tc.For_i(0, reps, 1):
                    emit_all()

    nc.compile()
    return nc


def _get_program(nb, L, reps=1):
    key = (nb, L, reps)
    if key not in _prog_cache:
        _prog_cache[key] = build_program(nb, L, reps=reps)
    return _prog_cache[key]


def kernel(query, context, mask, W_in, W_out):
    from concourse.bass_utils import run_bass_kernel_spmd

    query = np.ascontiguousarray(query, dtype=np.float16)
    context = np.ascontiguousarray(context, dtype=np.float16)
    W_in = np.ascontiguousarray(W_in, dtype=np.float16)
    W_out = np.ascontiguousarray(W_out, dtype=np.float16)
    B, L, _ = query.shape
    mask3 = np.ascontiguousarray(mask.reshape(B, L, -1), dtype=np.int32)

    nb = B // N_CORES
    nc = _get_program(nb, L)
    in_maps = []
    for c in range(N_CORES):
        b0 = c * nb
        in_maps.append({
            "query": query[b0:b0 + nb],
            "context": context[b0:b0 + nb],
            "mask": mask3[b0:b0 + nb],
            "W_in": W_in,
            "W_out": W_out,
        })
    res = run_bass_kernel_spmd(nc, in_maps, core_ids=list(range(N_CORES)))
    global last_results
    last_results = res
    out = np.concatenate([r["out"] for r in res.results], axis=0)
    return out
